# revision 20
# baseline (speedup 1.0000x reference)
"""Trainium2 Bass kernel for Autoformer-style autocorrelation attention.

Math (matches the reference nn.Module):
    top_k = int(log(L)) = 6
    mean_value[b, l] = corr[b].mean(over H, C)                     # [B, L]
    idx = top_k(mean_value.mean(over B))                           # [6]
    w = softmax(mean_value[:, idx], axis=-1)                       # [B, 6]
    out[b, h, c, l] = sum_k w[b, k] * values[b, h, c, (l+idx_k)%L]

Strategy: data-parallel over B (4 batches per core on 8 cores), two
launches with tiny host glue (top-k + softmax) in between.

Both launches use a host-permuted "P-major" DRAM layout
buf[b, p, t, l] = x[b, t*128 + p, l] so each SBUF partition's slice of a
batch is contiguous in DRAM (large DMA packets, one descriptor per
partition), streamed in 0.5 MiB chunks so compute starts ~1.5us after
the first trigger.

Launch 1 reduces corr (fp16; the ~1e-5 quantization error on the means
is far below the 1.1e-4 top-k selection margin; fp8/int8 are NOT safe)
over (H, C) per batch via ones-matmuls accumulated in PSUM across the
batch's 4 row-tiles.  Scalar-engine copies move psum row 0 to SBUF and
per-batch DMAs return the [B, L] sums to the host, which runs the tiny
top-k + softmax glue.  The vector engine is untouched.

Launch 2 computes the 6-shift weighted combine with shifts baked in as
static SBUF column windows.  Each 1024-col tile is host-extended with a
copy of its own first 512 cols so every 512-wide shifted window is one
contiguous matmul piece (+25% values DMA, which has slack; saves one
matmul per term on the PE).  Terms are split by measured engine rates
(PE ~0.53us vs DVE ~1.27us per term-tile): 4 terms per tile on the
tensor engine as diag(w_bk) @ shifted-window matmuls into PSUM (5 on
the last tile of each batch), the rest on the vector engine as
scalar_tensor_tensor chains over the scalar engine's PSUM->fp16
conversion; DVE terms are ordered to shifts <= 512 so their full-width
stt needs a single piece.  Per-batch weights enter as [128,1] AP
scalars from an input tensor so one NEFF is SPMD across all cores.
Output is fp16 written per half-batch (host upcasts; adds ~5e-4
relative, tolerance 2e-2), halving output DMA traffic.
"""

import math

import numpy as np

_B, _H, _C, _L = 32, 8, 64, 1024
_NCORES = 8
_BLOC = _B // _NCORES  # batches per core
_R = _H * _C           # rows per batch
_PART = 128
_TPB = _R // _PART     # SBUF row-tiles per batch (4)
_TOPK = int(math.log(_L))  # 6
_NPE = 4               # shift terms handled by the tensor engine
_HALF = 512            # PSUM bank width in fp32


def _build_phase1():
    import concourse.bacc as bacc
    import concourse.mybir as mybir
    import concourse.tile as tile

    f32 = mybir.dt.float32
    f16 = mybir.dt.float16
    act_copy = mybir.ActivationFunctionType.Copy
    nc = bacc.Bacc("TRN2", target_bir_lowering=False, debug=False,
                   enable_partition_id=False)
    # P-major: corr_sh[b, p, t*L + l] = corr[b, t*128 + p, l]
    corr_d = nc.dram_tensor("corr_sh", [_BLOC, _PART, _TPB * _L], f16,
                            kind="ExternalInput").ap()
    sums_d = nc.dram_tensor("sums", [1, _BLOC * _L], f32,
                            kind="ExternalOutput").ap()

    _CH = _TPB * _L   # chunk cols: one full batch (1 MiB) per DMA
    _NCH = _BLOC

    def chunk_dma(io_pool, c):
        vt = io_pool.tile([_PART, _CH], f16, tag="vt")
        nc.sync.dma_start(vt[:], corr_d[c])
        return vt

    with tile.TileContext(nc) as tc:
        with (
            tc.tile_pool(name="io", bufs=4) as io_pool,
            tc.tile_pool(name="const", bufs=1) as const_pool,
            tc.tile_pool(name="acc", bufs=1) as acc_pool,
            tc.tile_pool(name="ps", bufs=2, space="PSUM") as ps_pool,
        ):
            # queue ALL input DMAs upfront: the queue streams back-to-back at
            # full rate while compute consumes chunks as they land
            pending = [chunk_dma(io_pool, c) for c in range(_NCH)]
            ones = const_pool.tile([_PART, _HALF], f16)
            nc.vector.memset(ones[:], 1.0)
            outs = acc_pool.tile([1, _BLOC * _L], f32)
            # HAM warmup: junk matmuls bring the PE clock to 2.4 GHz while
            # the first chunk DMA is still in flight
            wps = ps_pool.tile([_PART, _HALF], f32, tag="wps", name="wps",
                               bufs=1)
            for _ in range(8):
                nc.tensor.matmul(wps[:], ones[:, 0:_PART], ones[:],
                                 start=True, stop=True)

            for b in range(_BLOC):
                pss = [ps_pool.tile([_PART, _HALF], f32, tag=f"ps{h}",
                                    name=f"ps{b}_{h}")
                       for h in range(2)]
                vt = pending[b]
                for t in range(_TPB):
                    for h in range(2):
                        nc.tensor.matmul(
                            pss[h][:],
                            ones[:, 0:_PART],
                            vt[:, t * _L + h * _HALF:
                               t * _L + (h + 1) * _HALF],
                            start=(t == 0),
                            stop=(t == _TPB - 1),
                        )
                for h in range(2):
                    o0 = b * _L + h * _HALF
                    nc.scalar.activation(outs[0:1, o0:o0 + _HALF],
                                         pss[h][0:1, :], act_copy)
                nc.scalar.dma_start(sums_d[0:1, b * _L:(b + 1) * _L],
                                    outs[0:1, b * _L:(b + 1) * _L])
    nc.compile()
    return nc


def _build_phase2(idx):
    import concourse.bacc as bacc
    import concourse.mybir as mybir
    import concourse.tile as tile

    f16 = mybir.dt.float16
    f32 = mybir.dt.float32
    alu = mybir.AluOpType
    act_copy = mybir.ActivationFunctionType.Copy

    nc = bacc.Bacc("TRN2", target_bir_lowering=False, debug=False,
                   enable_partition_id=False)
    # each tile extended with a copy of its own first 512 cols so every
    # 512-wide shifted window is ONE contiguous matmul piece
    _LE = _L + _HALF
    vals_d = nc.dram_tensor("vals", [_BLOC, _PART, _TPB * _LE], f16,
                            kind="ExternalInput").ap()
    wsb_d = nc.dram_tensor("wsb", [_PART, _BLOC * _TOPK], f32,
                           kind="ExternalInput").ap()
    diag_d = nc.dram_tensor("diags", [_PART, _BLOC * _TOPK * _PART], f16,
                            kind="ExternalInput").ap()
    out_d = nc.dram_tensor("out_sh", [_BLOC, _PART, _TPB * _L], f16,
                           kind="ExternalOutput").ap()

    _CH = 2 * _LE
    _NCH = _BLOC * 2

    with tile.TileContext(nc) as tc:
        with (
            tc.tile_pool(name="const", bufs=1) as const_pool,
            tc.tile_pool(name="v16", bufs=8) as v16_pool,
            tc.tile_pool(name="tmp", bufs=4) as tmp_pool,
            tc.tile_pool(name="out", bufs=2) as out_pool,
            tc.tile_pool(name="ps", bufs=3, space="PSUM") as ps_pool,
        ):
            # constants + ALL value chunks stream back-to-back from the start
            w_t = const_pool.tile([_PART, _BLOC * _TOPK], f32)
            nc.sync.dma_start(w_t[:], wsb_d[:])
            diag = const_pool.tile([_PART, _BLOC * _TOPK * _PART], f16)
            nc.sync.dma_start(diag[:], diag_d[:])
            vts = []
            for c in range(_NCH):
                b, u = divmod(c, 2)
                vt = v16_pool.tile([_PART, _CH], f16, tag="vt")
                nc.sync.dma_start(vt[:], vals_d[b, :, u * _CH:(u + 1) * _CH])
                vts.append(vt)

            # HAM warmup while the first DMAs land
            wones = const_pool.tile([_PART, _HALF], f16)
            nc.vector.memset(wones[:], 1.0)
            wps = ps_pool.tile([_PART, _HALF], f32, tag="wps", name="wps",
                               bufs=1)
            for _ in range(12):
                nc.tensor.matmul(wps[:], wones[:, 0:_PART], wones[:],
                                 start=True, stop=True)

            for b in range(_BLOC):
                ot = out_pool.tile([_PART, _TPB * _L], f16, tag="ot")
                for u in range(2):
                    vt = vts[b * 2 + u]
                    for t2 in range(2):
                        t = u * 2 + t2
                        c0 = t2 * _LE      # col window within chunk
                        o0 = t * _L        # col window within ot
                        # 4 PE terms per tile, 5 on the last tile of each
                        # batch: balances PE (~68 terms) vs DVE (~28) and
                        # makes the final chain a single stt (short trail)
                        npe = 5 if t == 3 else 4

                        pss = [ps_pool.tile([_PART, _HALF], f32,
                                            tag=f"ps{h}", name=f"ps{h}",
                                            bufs=4 if h == 0 else 3)
                               for h in range(2)]
                        for h in range(2):
                            for k in range(npe):
                                s = (idx[k] + h * _HALF) % _L
                                nc.tensor.matmul(
                                    pss[h][:],
                                    diag[:, (b * _TOPK + k) * _PART:
                                         (b * _TOPK + k + 1) * _PART],
                                    vt[:, c0 + s:c0 + s + _HALF],
                                    start=(k == 0),
                                    stop=(k == npe - 1),
                                )

                        # Scalar: PSUM -> fp16 SBUF conversion
                        x0 = tmp_pool.tile([_PART, _L], f16, tag="x0")
                        for h in range(2):
                            nc.scalar.activation(
                                x0[:, h * _HALF:(h + 1) * _HALF],
                                pss[h][:], act_copy)

                        def stt_term(dst, dstof, s, wap, src, vt=vt, c0=c0):
                            """dst[l] = w*vt_tile[(l+s)%L] + src[l]"""
                            m = min(_L, _LE - s)
                            nc.vector.scalar_tensor_tensor(
                                dst[:, dstof:dstof + m],
                                vt[:, c0 + s:c0 + s + m], wap,
                                src[:, 0:m],
                                op0=alu.mult, op1=alu.add)
                            if m < _L:
                                nc.vector.scalar_tensor_tensor(
                                    dst[:, dstof + m:dstof + _L],
                                    vt[:, c0 + _HALF:c0 + s], wap,
                                    src[:, m:_L],
                                    op0=alu.mult, op1=alu.add)

                        # DVE: remaining terms chained onto x0 -> ot
                        wk = lambda k: w_t[:, b * _TOPK + k:b * _TOPK + k + 1]
                        if npe == 5:
                            stt_term(ot, o0, idx[5], wk(5), x0)
                        else:
                            x1 = tmp_pool.tile([_PART, _L], f16, tag="x1")
                            stt_term(x1, 0, idx[4], wk(4), x0)
                            stt_term(ot, o0, idx[5], wk(5), x1)
                    # half-batch output DMA: starts write traffic earlier and
                    # halves the final post-compute transfer
                    nc.scalar.dma_start(
                        out_d[b, :, u * 2 * _L:(u + 1) * 2 * _L],
                        ot[:, u * 2 * _L:(u + 1) * 2 * _L])
    nc.compile()
    return nc


def _run_spmd(nc, in_maps, **kwargs):
    from concourse import bass_utils

    return bass_utils.run_bass_kernel_spmd(
        nc, in_maps, core_ids=list(range(_NCORES)), **kwargs
    )


def _pmajor(x):
    """[n, R, L] -> [n, 128, TPB*L] with buf[n, p, t*L+l] = x[n, t*128+p, l]."""
    n = x.shape[0]
    return np.ascontiguousarray(
        x.reshape(n, _TPB, _PART, _L).transpose(0, 2, 1, 3)
        .reshape(n, _PART, _TPB * _L))


def _unpmajor(x):
    """Inverse of _pmajor."""
    n = x.shape[0]
    return (x.reshape(n, _PART, _TPB, _L).transpose(0, 2, 1, 3)
            .reshape(n, _R, _L))


def kernel(values: np.ndarray, corr: np.ndarray, _collect=None) -> np.ndarray:
    assert values.shape == (_B, _H, _C, _L) and corr.shape == (_B, _H, _C, _L)
    corr16 = _pmajor(np.asarray(corr, dtype=np.float16).reshape(_B, _R, _L))
    vals16 = _pmajor(np.asarray(values, dtype=np.float16).reshape(_B, _R, _L))

    # ---- launch 1: per-batch sums of corr over (H, C) ----
    nc1 = _build_phase1()
    in1 = [
        {"corr_sh": corr16[c * _BLOC:(c + 1) * _BLOC]}
        for c in range(_NCORES)
    ]
    res1 = _run_spmd(nc1, in1, **(_collect.kwargs(1) if _collect else {}))
    if _collect is not None:
        _collect.add(1, nc1, res1)
    sums = np.concatenate(
        [r["sums"].reshape(_BLOC, _L) for r in res1.results], axis=0
    )  # [B, L]

    # ---- host glue: top-k indices + softmax weights (tiny) ----
    mean_value = sums / np.float32(_R)                       # [B, L]
    g = mean_value.astype(np.float64).mean(axis=0)           # [L]
    idx = np.argsort(-g, kind="stable")[:_TOPK].astype(np.int64)
    wsel = mean_value[:, idx].astype(np.float32)             # [B, 6]
    e = np.exp(wsel - wsel.max(axis=-1, keepdims=True))
    w = (e / e.sum(axis=-1, keepdims=True)).astype(np.float32)

    # ---- launch 2: weighted shifted-gather combine ----
    # order terms so the DVE terms (positions 4,5) have shift <= 512 when
    # possible (their full-width stt then needs a single piece)
    order = sorted(range(_TOPK), key=lambda k: int(idx[k]) <= _HALF)
    idx = idx[order]
    w = w[:, order]
    nc2 = _build_phase2([int(i) for i in idx])
    # extended P-major layout: tile cols [0:1024]=v, [1024:1536]=v[0:512]
    v4 = np.asarray(values, dtype=np.float16).reshape(_B, _TPB, _PART, _L)
    v4e = np.concatenate([v4, v4[..., :_HALF]], axis=-1)  # [B,TPB,128,1536]
    vals16e = np.ascontiguousarray(
        v4e.transpose(0, 2, 1, 3).reshape(_B, _PART, _TPB * (_L + _HALF)))
    eye = np.eye(_PART, dtype=np.float16)
    in2 = []
    for c in range(_NCORES):
        wloc = w[c * _BLOC:(c + 1) * _BLOC]                  # [BLOC, 6]
        wsb = np.ascontiguousarray(
            np.broadcast_to(
                wloc.reshape(-1)[None, :], (_PART, _BLOC * _TOPK)),
            dtype=np.float32,
        )
        diags = np.concatenate(
            [eye * np.float16(wloc[b, k]) for b in range(_BLOC)
             for k in range(_TOPK)],
            axis=1,
        )  # [128, BLOC*TOPK*128] fp16
        in2.append({
            "vals": vals16e[c * _BLOC:(c + 1) * _BLOC],
            "wsb": wsb,
            "diags": np.ascontiguousarray(diags),
        })
    res2 = _run_spmd(nc2, in2, **(_collect.kwargs(2) if _collect else {}))
    if _collect is not None:
        _collect.add(2, nc2, res2)
    out = np.concatenate([_unpmajor(r["out_sh"]) for r in res2.results],
                         axis=0).astype(np.float32)
    return out.reshape(_B, _H, _C, _L)
